# revision 1
# baseline (speedup 1.0000x reference)
"""Trainium2 Bass kernel for nn_DependencyBertMix.

Contract: kernel(**inputs) takes the FULL unsharded inputs (as produced by
setup_inputs()) and returns the FULL [8, 512, 768] float32 output.

Strategy: data-parallel over batch — B=8 batch elements, one per NeuronCore.
Weights are replicated to all 8 cores; no collectives.

Per-core pipeline (kept in a "transposed" [feature, t] layout so no on-chip
transposes are needed; t = query position, s = key position, both 512):

  Q_T[co,t] = Wq^T @ hidden_T       K_T likewise       V[t,co] = hidden_T^T @ Wv
  per head h (12 heads, d=64; heads processed in cotile-pairs):
    S_T[s,t]   = K_h-slices as lhsT @ Q_h            (scores transposed)
    A_T        = S_T/8 (+ mask bias)                 (self_attn^T, bf16)
    D_T        = S_T * (dep^T/8)                     (dep_self_attn^T, bf16)
    stats      : column sums over s of A,D and squares via ones-matmuls
                 (replicated rows), var/rsqrt on-chip
    LayerNorm + gating MLP folded into matmuls:
      Wg = W1 * ln_g[:,None];  s_vec = colsum(Wg);  c_vec = ln_b@W1 + b1
      Y[n,t]   = Wg^T @ [A;D] - s_vec x mu + c_vec x sqrt(var+eps)
      tanh_T   = tanh(rs * Y)
      G[n2,t]  = W2^T @ tanh_T (+ b2);  g_T = sigmoid(G)
    mixed      = g*A + (1-g)*D ;  E = exp(mixed)     (no max-sub, |mixed|<~6)
    den[t]     = E^T-chunks @ ones  (column form)    rden = 1/den  (cheap)
    ctx[t,d]   = E^T-chunks @ V_h, * rden            -> natural [T,C] output

Matmul operands are bfloat16 (fp32 accumulate in PSUM); the LN-fold rows
(negs/c x mu/sqv) run as separate float32r matmuls for precision.
"""

import sys

for _p in ("/opt/trn_rl_repo", "/opt/pypackages"):
    if _p not in sys.path:
        sys.path.append(_p)

import ml_dtypes
import numpy as np

B, T, C = 8, 512, 768
H, DH = 12, 64
TM = 512
EPS = 1e-5
N_CORES = 8
P = 128


def _build(flags):
    import concourse.tile as tile
    from concourse import bacc, mybir

    f32 = mybir.dt.float32
    f32r = mybir.dt.float32r
    bf16 = mybir.dt.bfloat16
    AF = mybir.ActivationFunctionType
    OP = mybir.AluOpType

    nc = bacc.Bacc("TRN2", target_bir_lowering=False, debug=False,
                   enable_asserts=False, num_devices=N_CORES)

    # ---- DRAM I/O (host-prepared layouts; weights pre-cast to bf16) ----
    hid_t = nc.dram_tensor("hid_t", [C, T], bf16, kind="ExternalInput")
    dep_t = nc.dram_tensor("dep_t", [T, T], f32, kind="ExternalInput")  # dep^T/8
    wq = nc.dram_tensor("wq", [C, C], bf16, kind="ExternalInput")
    wk = nc.dram_tensor("wk", [C, C], bf16, kind="ExternalInput")
    wv = nc.dram_tensor("wv", [C, C], bf16, kind="ExternalInput")
    w1 = nc.dram_tensor("w1", [2 * TM, TM], bf16, kind="ExternalInput")
    w2 = nc.dram_tensor("w2", [TM, TM], bf16, kind="ExternalInput")
    out_t = nc.dram_tensor("out_t", [T, C], f32, kind="ExternalOutput")

    bq_d = nc.dram_tensor("bq", [C], f32, kind="ExternalInput") if flags["bq"] else None
    bk_d = nc.dram_tensor("bk", [C], f32, kind="ExternalInput") if flags["bk"] else None
    bv_d = (nc.dram_tensor("bv", [C], bf16, kind="ExternalInput")
            if flags["bv"] else None)
    lng_d = (nc.dram_tensor("lng", [2 * TM], f32, kind="ExternalInput")
             if flags["lng"] else None)
    if flags["c"]:
        lnb_d = nc.dram_tensor("lnb", [2 * TM], bf16, kind="ExternalInput")
        b1_d = nc.dram_tensor("b1", [TM], f32, kind="ExternalInput")
    b2_d = (nc.dram_tensor("b2", [TM], bf16, kind="ExternalInput")
            if flags["b2"] else None)
    mb_d = (nc.dram_tensor("mb", [T], f32, kind="ExternalInput")
            if flags["mask"] else None)

    CI = C // P   # 6
    CO = C // P   # 6
    ST = T // P   # 4
    TT = T // P   # 4
    KT8 = 2 * TM // P  # 8
    NT = TM // P  # 4

    with tile.TileContext(nc) as tc:
        with (
            tc.tile_pool(name="singles", bufs=1) as singles,
            tc.tile_pool(name="wpool", bufs=3) as wpool,
            tc.tile_pool(name="adpool", bufs=36) as adpool,
            tc.tile_pool(name="sqpool", bufs=4) as sqpool,
            tc.tile_pool(name="stpool", bufs=8) as stpool,
            tc.tile_pool(name="mixpool", bufs=8) as mixpool,
            tc.tile_pool(name="mixbpool", bufs=16) as mixbpool,
            tc.tile_pool(name="thpool", bufs=8) as thpool,
            tc.tile_pool(name="gpool", bufs=8) as gpool,
            tc.tile_pool(name="epool", bufs=12) as epool,
            tc.tile_pool(name="rdpool", bufs=4) as rdpool,
            tc.tile_pool(name="opool", bufs=10) as opool,
            tc.tile_pool(name="pspool", bufs=8, space="PSUM") as pspool,
        ):
            def ps_tile():
                return pspool.tile([P, 512], f32, tag="ps", name="ps")

            # ---------- persistent loads ----------
            hid_sb = singles.tile([P, CI, T], bf16)
            for ci in range(CI):
                nc.sync.dma_start(out=hid_sb[:, ci, :],
                                  in_=hid_t[ci * P:(ci + 1) * P, :])
            dep_sb = singles.tile([P, ST, T], f32)
            for j in range(ST):
                nc.sync.dma_start(out=dep_sb[:, j, :],
                                  in_=dep_t[j * P:(j + 1) * P, :])

            ones_b = singles.tile([P, P], bf16)
            nc.vector.memset(ones_b[:], 1.0)
            eps_col = singles.tile([P, 1], f32)
            nc.vector.memset(eps_col[:], EPS)

            wg_sb = singles.tile([P, KT8, TM], bf16)
            for j in range(KT8):
                nc.sync.dma_start(out=wg_sb[:, j, :], in_=w1[j * P:(j + 1) * P, :])
            w2_sb = singles.tile([P, NT, TM], bf16)
            for j in range(NT):
                nc.sync.dma_start(out=w2_sb[:, j, :], in_=w2[j * P:(j + 1) * P, :])

            c_sb = None
            if flags["c"]:
                lnb_sb = singles.tile([P, KT8], bf16)
                nc.sync.dma_start(out=lnb_sb[:],
                                  in_=lnb_d[:].rearrange("(j p) -> p j", p=P))
                b1_sb = singles.tile([1, TM], f32)
                nc.sync.dma_start(out=b1_sb[:], in_=b1_d[None, :])
                c_ps = ps_tile()
                for j in range(KT8):
                    nc.tensor.matmul(c_ps[0:1, :], lhsT=lnb_sb[:, j:j + 1],
                                     rhs=wg_sb[:, j, :],
                                     start=(j == 0), stop=(j == KT8 - 1))
                c_sb = singles.tile([1, TM], f32r)
                nc.vector.tensor_add(c_sb[:], c_ps[0:1, :], b1_sb[:])

            if flags["lng"]:
                lng_sb = singles.tile([P, KT8], f32)
                nc.sync.dma_start(out=lng_sb[:],
                                  in_=lng_d[:].rearrange("(j p) -> p j", p=P))
                for j in range(KT8):
                    nc.vector.tensor_scalar_mul(wg_sb[:, j, :], wg_sb[:, j, :],
                                                lng_sb[:, j:j + 1])

            # s_vec = column sums of Wg (post ln_g fold); store negated f32r row
            s_ps = ps_tile()
            for j in range(KT8):
                nc.tensor.matmul(s_ps[0:1, :], lhsT=ones_b[:, 0:1],
                                 rhs=wg_sb[:, j, :],
                                 start=(j == 0), stop=(j == KT8 - 1))
            negs_sb = singles.tile([1, TM], f32r)
            nc.scalar.activation(negs_sb[:], s_ps[0:1, :], AF.Copy, scale=-1.0)

            b2_sb = None
            if flags["b2"]:
                b2_sb = singles.tile([1, TM], bf16)
                nc.sync.dma_start(out=b2_sb[:], in_=b2_d[None, :])
            bv_sb = None
            if flags["bv"]:
                bv_sb = singles.tile([1, C], bf16)
                nc.sync.dma_start(out=bv_sb[:], in_=bv_d[None, :])
            bq_sb = None
            if flags["bq"]:
                bq_sb = singles.tile([P, CO], f32)
                nc.sync.dma_start(out=bq_sb[:],
                                  in_=bq_d[:].rearrange("(j p) -> p j", p=P))
            bk_sb = None
            if flags["bk"]:
                bk_sb = singles.tile([P, CO], f32)
                nc.sync.dma_start(out=bk_sb[:],
                                  in_=bk_d[:].rearrange("(j p) -> p j", p=P))
            mb_sb = None
            if flags["mask"]:
                mb_sb = singles.tile([P, ST], f32)
                nc.sync.dma_start(out=mb_sb[:],
                                  in_=mb_d[:].rearrange("(j p) -> p j", p=P))

            # ---------- QKV projections ----------
            QT = [singles.tile([P, T], bf16, tag=f"qt{i}", name=f"qt{i}")
                  for i in range(CO)]
            KTt = [singles.tile([P, T], bf16, tag=f"kt{i}", name=f"kt{i}")
                   for i in range(CO)]
            # V augmented with a ones column per head: [128, 12 heads, 65]
            vaug = [singles.tile([P, H, DH + 1], bf16, tag=f"v{i}", name=f"v{i}")
                    for i in range(TT)]
            for tt in range(TT):
                nc.vector.memset(vaug[tt][:, :, DH:DH + 1], 1.0)

            for wdram, dest, bsb in ((wq, QT, bq_sb), (wk, KTt, bk_sb)):
                ps_l = [ps_tile() for _ in range(CO)]
                for ci in range(CI):
                    w_ci = wpool.tile([P, C], bf16, tag="w", name="w")
                    nc.sync.dma_start(out=w_ci[:], in_=wdram[ci * P:(ci + 1) * P, :])
                    for cot in range(CO):
                        nc.tensor.matmul(ps_l[cot][:],
                                         lhsT=w_ci[:, cot * P:(cot + 1) * P],
                                         rhs=hid_sb[:, ci, :],
                                         start=(ci == 0), stop=(ci == CI - 1))
                for cot in range(CO):
                    if bsb is not None:
                        nc.scalar.activation(dest[cot][:], ps_l[cot][:], AF.Identity,
                                             bias=bsb[:, cot:cot + 1])
                    else:
                        nc.scalar.copy(dest[cot][:], ps_l[cot][:])

            NCH = 2
            CHW = C // NCH  # 384
            v_ps = [[pspool.tile([P, CHW], f32, tag="ps", name="vps")
                     for _ in range(NCH)] for _ in range(TT)]
            last_v = CI - 1 if not flags["bv"] else None
            for ci in range(CI):
                w_ci = wpool.tile([P, C], bf16, tag="w", name="w")
                nc.sync.dma_start(out=w_ci[:], in_=wv[ci * P:(ci + 1) * P, :])
                for tt in range(TT):
                    for ch in range(NCH):
                        nc.tensor.matmul(
                            v_ps[tt][ch][:],
                            lhsT=hid_sb[:, ci, tt * P:(tt + 1) * P],
                            rhs=w_ci[:, ch * CHW:(ch + 1) * CHW],
                            start=(ci == 0), stop=(ci == last_v))
            HPC = CHW // DH  # 6 heads per chunk
            for tt in range(TT):
                for ch in range(NCH):
                    if flags["bv"]:
                        nc.tensor.matmul(v_ps[tt][ch][:],
                                         lhsT=ones_b[0:1, :],
                                         rhs=bv_sb[:, ch * CHW:(ch + 1) * CHW],
                                         start=False, stop=True)
                    for hh in range(HPC):
                        nc.scalar.copy(vaug[tt][:, ch * HPC + hh, 0:DH],
                                       v_ps[tt][ch][:, hh * DH:(hh + 1) * DH])

            # ---------- per-head pipeline, cotile pairs ----------
            inv2t = 1.0 / (2 * TM)

            def scores_phase(h):
                cot, poff = h // 2, (h % 2) * DH
                q_h = QT[cot][poff:poff + DH, :]
                k_h = KTt[cot][poff:poff + DH, :]
                A = [adpool.tile([P, T], bf16, tag="ad", name="A")
                     for _ in range(ST)]
                D = [adpool.tile([P, T], bf16, tag="ad", name="D")
                     for _ in range(ST)]
                diffs = []
                for j in range(ST):
                    sp = ps_tile()
                    nc.tensor.matmul(sp[:], lhsT=k_h[:, j * P:(j + 1) * P],
                                     rhs=q_h, start=True, stop=True)
                    if flags["mask"]:
                        nc.scalar.activation(A[j][:], sp[:], AF.Identity,
                                             scale=0.125, bias=mb_sb[:, j:j + 1])
                    else:
                        nc.scalar.activation(A[j][:], sp[:], AF.Copy, scale=0.125)
                    nc.vector.tensor_mul(D[j][:], sp[:], dep_sb[:, j, :])
                    diff = mixbpool.tile([P, T], bf16, tag="mixb", name="diff")
                    nc.vector.tensor_sub(diff[:], A[j][:], D[j][:])
                    diffs.append(diff)
                return A, D, diffs

            def stats_phase(A, D):
                mu_ps = ps_tile()
                for j, src in enumerate(A + D):
                    nc.tensor.matmul(mu_ps[:], lhsT=ones_b[:], rhs=src[:],
                                     start=(j == 0), stop=(j == 2 * ST - 1))
                ms_ps = ps_tile()
                for j, src in enumerate(A + D):
                    sq = sqpool.tile([P, T], bf16, tag="sq", name="sq")
                    nc.gpsimd.tensor_mul(sq[:], src[:], src[:])
                    nc.tensor.matmul(ms_ps[:], lhsT=ones_b[:], rhs=sq[:],
                                     start=(j == 0), stop=(j == 2 * ST - 1))
                mu_rep = stpool.tile([P, T], f32r, tag="st", name="mu_rep")
                nc.scalar.activation(mu_rep[:], mu_ps[:], AF.Copy, scale=inv2t)
                mu2 = mixpool.tile([P, T], f32, tag="mix", name="mu2")
                nc.vector.tensor_mul(mu2[:], mu_rep[:], mu_rep[:])
                var = mixpool.tile([P, T], f32, tag="mix", name="var")
                nc.vector.scalar_tensor_tensor(var[:], ms_ps[:], inv2t, mu2[:],
                                               op0=OP.mult, op1=OP.subtract)
                return mu_rep, var

            def rs_phase(var):
                sqv_rep = stpool.tile([P, T], f32r, tag="st", name="sqv_rep")
                nc.scalar.activation(sqv_rep[:], var[:], AF.Sqrt, bias=eps_col[:])
                rs_rep = stpool.tile([P, T], f32, tag="st", name="rs_rep")
                nc.vector.reciprocal(rs_rep[:], sqv_rep[:])
                return sqv_rep, rs_rep

            def mlp1_phase(A, D, mu_rep, sqv_rep, rs_rep):
                th_l = []
                ti_l = []
                for nt in range(NT):
                    y_ps = ps_tile()
                    nsl = slice(nt * P, (nt + 1) * P)
                    for j, src in enumerate(A + D):
                        nc.tensor.matmul(y_ps[:], lhsT=wg_sb[:, j, nsl],
                                         rhs=src[:], start=(j == 0), stop=False)
                    nc.tensor.matmul(y_ps[:], lhsT=negs_sb[0:1, nsl],
                                     rhs=mu_rep[0:1, :],
                                     start=False, stop=not flags["c"])
                    if flags["c"]:
                        nc.tensor.matmul(y_ps[:], lhsT=c_sb[0:1, nsl],
                                         rhs=sqv_rep[0:1, :],
                                         start=False, stop=True)
                    ti = mixpool.tile([P, T], f32, tag="mix", name="ti")
                    nc.vector.tensor_mul(ti[:], y_ps[:], rs_rep[:])
                    ti_l.append(ti)
                return ti_l

            def tanh_phase(ti_l):
                th_l = []
                for ti in ti_l:
                    th = thpool.tile([P, T], bf16, tag="th", name="th")
                    th_l.append(th)
                    nc.scalar.activation(th[:], ti[:], AF.Tanh)
                return th_l

            def mlp2_phase(th_l):
                g_ps_l = []
                for nt in range(NT):
                    g_ps = ps_tile()
                    nsl = slice(nt * P, (nt + 1) * P)
                    for j in range(NT):
                        nc.tensor.matmul(g_ps[:], lhsT=w2_sb[:, j, nsl],
                                         rhs=th_l[j][:], start=(j == 0),
                                         stop=(j == NT - 1 and not flags["b2"]))
                    if flags["b2"]:
                        nc.tensor.matmul(g_ps[:], lhsT=b2_sb[0:1, nsl],
                                         rhs=ones_b[0:1, :],
                                         start=False, stop=True)
                    g_ps_l.append(g_ps)
                return g_ps_l

            def sigmoid_phase(g_ps_l):
                g_l = []
                for g_ps in g_ps_l:
                    gt = gpool.tile([P, T], bf16, tag="g", name="gt")
                    g_l.append(gt)
                    nc.scalar.activation(gt[:], g_ps[:], AF.Sigmoid)
                return g_l

            def mix_phase(D, diffs, g_l):
                mixd_l = []
                for j in range(ST):
                    prod = mixbpool.tile([P, T], bf16, tag="mixb", name="prod")
                    nc.vector.tensor_mul(prod[:], g_l[j][:], diffs[j][:])
                    mixd = mixbpool.tile([P, T], bf16, tag="mixb", name="mixd")
                    nc.gpsimd.tensor_add(mixd[:], prod[:], D[j][:])
                    mixd_l.append(mixd)
                return mixd_l

            def exp_phase(mixd_l):
                E_l = []
                for mixd in mixd_l:
                    E = epool.tile([P, T], bf16, tag="e", name="E")
                    E_l.append(E)
                    nc.scalar.activation(E[:], mixd[:], AF.Exp)
                return E_l

            def ctx_phase(h, E_l):
                # one psum bank: [ctx|den] chunks of width 65 at cols c*65
                W65 = DH + 1
                dc = ps_tile()
                for cch in range(4):
                    for j in range(ST):
                        nc.tensor.matmul(
                            dc[:, cch * W65:(cch + 1) * W65],
                            lhsT=E_l[j][:, cch * P:(cch + 1) * P],
                            rhs=vaug[j][:, h, :],
                            start=(j == 0), stop=(j == ST - 1))
                rden = rdpool.tile([P, 4], f32, tag="rd", name="rden")
                for cch in range(4):
                    nc.vector.reciprocal(rden[:, cch:cch + 1],
                                         dc[:, cch * W65 + DH:cch * W65 + DH + 1])
                for cch in range(4):
                    ctxn = opool.tile([P, DH], f32, tag="o", name="ctxn")
                    nc.vector.tensor_scalar_mul(ctxn[:], dc[:, cch * W65:cch * W65 + DH],
                                                rden[:, cch:cch + 1])
                    nc.sync.dma_start(
                        out=out_t[cch * P:(cch + 1) * P, h * DH:(h + 1) * DH],
                        in_=ctxn[:])

            for pc in range(H // 2):
                h0, h1 = 2 * pc, 2 * pc + 1
                A0, D0, df0 = scores_phase(h0)
                A1, D1, df1 = scores_phase(h1)
                mu0, var0 = stats_phase(A0, D0)
                mu1, var1 = stats_phase(A1, D1)
                sqv0, rs0 = rs_phase(var0)
                sqv1, rs1 = rs_phase(var1)
                ti0 = mlp1_phase(A0, D0, mu0, sqv0, rs0)
                ti1 = mlp1_phase(A1, D1, mu1, sqv1, rs1)
                th0 = tanh_phase(ti0)
                th1 = tanh_phase(ti1)
                gp0 = mlp2_phase(th0)
                gp1 = mlp2_phase(th1)
                g0 = sigmoid_phase(gp0)
                g1 = sigmoid_phase(gp1)
                mx0 = mix_phase(D0, df0, g0)
                mx1 = mix_phase(D1, df1, g1)
                E0 = exp_phase(mx0)
                E1 = exp_phase(mx1)
                ctx_phase(h0, E0)
                ctx_phase(h1, E1)

    nc.compile()
    return nc


def _prep(inputs):
    bfloat16 = ml_dtypes.bfloat16
    hidden = np.asarray(inputs["hidden_states"], dtype=np.float32)
    mask = np.asarray(inputs["attention_mask"], dtype=np.float32)
    dep = np.asarray(inputs["dependency_matrix"], dtype=np.float32)
    ws = {k: np.ascontiguousarray(
            np.asarray(inputs[k], dtype=np.float32).astype(bfloat16))
          for k in ("Wq", "Wk", "Wv", "W1", "W2")}
    vs = {k: np.asarray(inputs[k], dtype=np.float32)
          for k in ("bq", "bk", "bv", "b1", "b2", "ln_g", "ln_b")}
    mb = (1.0 - mask) * -10000.0

    flags = {
        "bq": bool(np.any(vs["bq"])), "bk": bool(np.any(vs["bk"])),
        "bv": bool(np.any(vs["bv"])),
        "lng": bool(np.any(vs["ln_g"] != 1.0)),
        "c": bool(np.any(vs["ln_b"]) or np.any(vs["b1"])),
        "b2": bool(np.any(vs["b2"])),
        "mask": bool(np.any(mb)),
    }

    in_maps = []
    for b in range(N_CORES):
        m = {
            "hid_t": np.ascontiguousarray(hidden[b].T.astype(bfloat16)),
            "dep_t": np.ascontiguousarray(dep[b].T * np.float32(0.125)),
            "wq": ws["Wq"], "wk": ws["Wk"], "wv": ws["Wv"],
            "w1": ws["W1"], "w2": ws["W2"],
        }
        if flags["bq"]:
            m["bq"] = vs["bq"]
        if flags["bk"]:
            m["bk"] = vs["bk"]
        if flags["bv"]:
            m["bv"] = vs["bv"].astype(bfloat16)
        if flags["lng"]:
            m["lng"] = vs["ln_g"]
        if flags["c"]:
            m["lnb"] = vs["ln_b"].astype(bfloat16)
            m["b1"] = vs["b1"]
        if flags["b2"]:
            m["b2"] = vs["b2"].astype(bfloat16)
        if flags["mask"]:
            m["mb"] = np.ascontiguousarray(mb[b])
        in_maps.append(m)
    return flags, in_maps


def kernel(**inputs):
    from concourse.bass_utils import run_bass_kernel_spmd

    flags, in_maps = _prep(inputs)
    nc = _build(flags)
    res = run_bass_kernel_spmd(nc, in_maps, core_ids=list(range(N_CORES)))
    out = np.stack([r["out_t"] for r in res.results])
    return out.astype(np.float32)

